# revision 1
# baseline (speedup 1.0000x reference)
"""Trainium2 Bass kernel: depthwise (per-sample, per-channel) 15x15 'same'
true convolution of 1024x3 images of 128x128, data-parallel over 8 NeuronCores.

Formulation (per (bn,c) pair, P=128, K=15, pad=7):
    out[y,x] = sum_{dy,dx} Xpad[y+dy, x+dx] * W[dy,dx],   W = flip(kernel)
y is split into 2 blocks of 64 rows. For block y0 and each dx the contribution
is a matmul with a banded block-Toeplitz stationary operand
    T[i, dx, j] = W[i-j, dx]  (i in 0..77, j in 0..63, band 0 <= i-j < 15)
    out[y0+j, x] += sum_i T[i, dx, j] * Xpad[y0+i, x+dx]
accumulated over the 15 dx values in PSUM. The two y-blocks write PSUM
partitions 0..63 / 64..127 (128x64 column tiling of the PE array). Operands
are fp16 (PSUM accumulates fp32); T and the padded images are prepared
host-side. Sharding: pure data parallel over BN (128 samples x 3 channels
= 384 independent images per core).
"""
import sys

sys.path.insert(0, "/opt/trn_rl_repo")

import numpy as np

_N_CORES = 8
_BN, _C, _P, _K = 1024, 3, 128, 15
_PAIRS_PER_CORE = (_BN // _N_CORES) * _C  # 384

_nc_cache = {}


def _build_nc(n_pairs: int, bufs: int = 6, psum_bufs: int = 4):
    import concourse.bacc as bacc
    import concourse.mybir as mybir
    from concourse import tile

    FP16 = mybir.dt.float16
    FP32 = mybir.dt.float32

    nc = bacc.Bacc("TRN2", target_bir_lowering=False, debug=False)
    xpad_d = nc.dram_tensor("xpad", [n_pairs, 142, 142], FP16, kind="ExternalInput")
    toep_d = nc.dram_tensor("toep", [n_pairs, 78, 15, 64], FP16, kind="ExternalInput")
    out_d = nc.dram_tensor("out", [n_pairs, 128, 128], FP32, kind="ExternalOutput")

    with tile.TileContext(nc) as tc:
        with (
            tc.tile_pool(name="xa", bufs=bufs) as xa_pool,
            tc.tile_pool(name="xb", bufs=bufs) as xb_pool,
            tc.tile_pool(name="tt", bufs=bufs) as tt_pool,
            tc.tile_pool(name="ot", bufs=bufs) as ot_pool,
            tc.tile_pool(name="ps", bufs=psum_bufs, space="PSUM") as ps_pool,
        ):
            for p in range(n_pairs):
                xa = xa_pool.tile([78, 142], FP16, tag="xa")
                xb = xb_pool.tile([78, 142], FP16, tag="xb")
                tt = tt_pool.tile([78, 15, 64], FP16, tag="tt")
                nc.sync.dma_start(out=xa[:], in_=xpad_d[p, 0:78, :])
                nc.sync.dma_start(out=xb[:], in_=xpad_d[p, 64:142, :])
                nc.sync.dma_start(out=tt[:], in_=toep_d[p])

                ps0 = ps_pool.tile([128, 128], FP32, tag="ps0")
                ps1 = ps_pool.tile([128, 128], FP32, tag="ps1")
                for dx in range(15):
                    lhsT = tt[:, dx, :]
                    nc.tensor.matmul(
                        ps0[0:64, :], lhsT, xa[:, dx:dx + 128],
                        start=(dx == 0), stop=(dx == 14),
                    )
                    nc.tensor.matmul(
                        ps1[64:128, :], lhsT, xb[:, dx:dx + 128],
                        start=(dx == 0), stop=(dx == 14),
                    )

                ot = ot_pool.tile([128, 128], FP32, tag="ot")
                nc.vector.tensor_copy(ot[0:64, :], ps0[0:64, :])
                nc.scalar.copy(ot[64:128, :], ps1[64:128, :])
                nc.sync.dma_start(out=out_d[p], in_=ot[:])

    nc.compile()
    return nc


def _host_prep(patches_pairs: np.ndarray, kernels_pairs: np.ndarray):
    """[NP,128,128] f32, [NP,15,15] f32 -> xpad [NP,142,142] fp16,
    toep [NP,78,15,64] fp16 with toep[p,i,dx,j] = flip(kern)[i-j, dx]."""
    NP = patches_pairs.shape[0]
    Xp = np.zeros((NP, 142, 142), dtype=np.float16)
    Xp[:, 7:135, 7:135] = patches_pairs.astype(np.float16)
    W = kernels_pairs[:, ::-1, ::-1].astype(np.float16)
    H = np.zeros((NP, 141, 15), dtype=np.float16)
    H[:, 63:78, :] = W
    s0, s1, s2 = H.strides
    A = np.lib.stride_tricks.as_strided(
        H[:, 63:, :], shape=(NP, 78, 64, 15), strides=(s0, s1, -s1, s2)
    )
    T = np.ascontiguousarray(A.transpose(0, 1, 3, 2))
    return Xp, T


def kernel(patches, kernels, kernel_size, patch_size, fft_size, _collect_results=None):
    """Full inputs in, full output out. Shards BN across 8 cores."""
    from concourse.bass_utils import run_bass_kernel_spmd

    patches = np.asarray(patches)
    kernels = np.asarray(kernels)
    assert patches.shape == (_BN, _C, _P, _P), patches.shape
    assert kernels.shape == (_BN, _C, _K, _K), kernels.shape

    if "nc" not in _nc_cache:
        _nc_cache["nc"] = _build_nc(_PAIRS_PER_CORE)
    nc = _nc_cache["nc"]

    bn_per_core = _BN // _N_CORES
    in_maps = []
    for core in range(_N_CORES):
        sl = slice(core * bn_per_core, (core + 1) * bn_per_core)
        pp = patches[sl].reshape(-1, _P, _P)
        kp = kernels[sl].reshape(-1, _K, _K)
        xpad, toep = _host_prep(pp, kp)
        in_maps.append({"xpad": xpad, "toep": toep})

    res = run_bass_kernel_spmd(nc, in_maps, core_ids=list(range(_N_CORES)))
    if _collect_results is not None:
        _collect_results.append(res)

    out = np.empty((_BN, _C, _P, _P), dtype=np.float32)
    for core in range(_N_CORES):
        sl = slice(core * bn_per_core, (core + 1) * bn_per_core)
        out[sl] = res.results[core]["out"].reshape(bn_per_core, _C, _P, _P)
    return out



# revision 2
# speedup vs baseline: 2.7410x; 2.7410x over previous
"""Trainium2 Bass kernel: depthwise (per-sample, per-channel) 15x15 'same'
true convolution of 1024x3 images of 128x128, data-parallel over 8 NeuronCores.

Formulation (per (bn,c) "job", P=128, K=15, pad=7):
    out[y,x] = sum_{dy,dx} Xpad[y+dy, x+dx] * W[dy,dx],   W = flip(kernel)
The 128 output rows are split into 8 blocks of 16. Per block and dx, the
contribution is a small banded-Toeplitz matmul
    T_dx[i, j] = W[i-j, dx]  (i in 0..29, j in 0..15, band 0 <= i-j < 15)
    out[16b+j, x] += sum_i T_dx[i, j] * Xpad[16b+i, x+dx]
accumulated over the 15 dx values in PSUM. These [K=30, M=16, N=512] matmuls
run on the PE array reconfigured as 16 concurrent 32x32 tiles
(tile_position): tile (r, c) streams from SBUF partitions 32r..32r+29 and
writes PSUM partitions 32c..32c+15 of bank r. One "set" = 8 jobs in flight
(each job -> 2 tile slots x 4 blocks in the rhs free dim, N = 4*128).
Operands are fp16 (PSUM accumulates fp32), output fp16. Host prepares the
SBUF-layout operands; sharding is pure data parallel over BN (128 samples
x 3 channels = 384 jobs = 48 sets per core).
"""
import sys

sys.path.insert(0, "/opt/trn_rl_repo")

import numpy as np

_N_CORES = 8
_BN, _C, _P, _K = 1024, 3, 128, 15
_PAIRS_PER_CORE = (_BN // _N_CORES) * _C  # 384
_SETS_PER_CORE = _PAIRS_PER_CORE // 8  # 48

_nc_cache = {}


def _build_nc(n_sets: int):
    import concourse.bacc as bacc
    import concourse.mybir as mybir
    from concourse import tile

    FP16 = mybir.dt.float16
    FP32 = mybir.dt.float32

    nc = bacc.Bacc("TRN2", target_bir_lowering=False, debug=False)
    xprep_d = nc.dram_tensor("xprep", [n_sets, 128, 4, 4, 142], FP16, kind="ExternalInput")
    tprep_d = nc.dram_tensor("tprep", [n_sets, 128, 2, 15, 16], FP16, kind="ExternalInput")
    out_d = nc.dram_tensor("out", [n_sets, 4, 16, 4, 4, 128], FP16, kind="ExternalOutput")

    with tile.TileContext(nc) as tc:
        with (
            tc.tile_pool(name="xp", bufs=3) as x_pool,
            tc.tile_pool(name="tp", bufs=3) as t_pool,
            tc.tile_pool(name="op", bufs=3) as o_pool,
            tc.tile_pool(name="ps", bufs=2, space="PSUM") as ps_pool,
        ):
            for s in range(n_sets):
                xt = x_pool.tile([128, 4, 4, 142], FP16, tag="xt")
                tt = t_pool.tile([128, 2, 15, 16], FP16, tag="tt")
                nc.sync.dma_start(out=xt[:], in_=xprep_d[s])
                nc.sync.dma_start(out=tt[:], in_=tprep_d[s])

                ps = [
                    ps_pool.tile([128, 4, 128], FP32, tag=f"ps{r}", name=f"ps{r}")
                    for r in range(4)
                ]

                for dx in range(15):
                    for r in range(4):
                        for c in range(4):
                            nc.tensor.matmul(
                                ps[r][32 * c:32 * c + 16, :, :],
                                tt[32 * r:32 * r + 30, c % 2, dx, :],
                                xt[32 * r:32 * r + 30, c, :, dx:dx + 128],
                                start=(dx == 0), stop=(dx == 14),
                                tile_position=(32 * r, 32 * c),
                            )

                ot = o_pool.tile([128, 4, 4, 128], FP16, tag="ot")
                for r in range(4):
                    eng = nc.vector.tensor_copy if r % 2 == 0 else nc.scalar.copy
                    eng(ot[:, r, :, :], ps[r][:, :, :])
                for c in range(4):
                    nc.sync.dma_start(out=out_d[s, c], in_=ot[32 * c:32 * c + 16, :, :, :])

    nc.compile()
    return nc


def _host_prep(patches_pairs: np.ndarray, kernels_pairs: np.ndarray):
    """[NJ,128,128] f32, [NJ,15,15] f32 -> xprep [S,128,4,4,142] fp16,
    tprep [S,128,2,15,16] fp16 laid out for the 16 PE tile slots.
    Job (s,r,cc) = jobs[8s + 4cc + r]; slot (r,c) holds blocks (c//2)*4..+3."""
    NJ = patches_pairs.shape[0]
    S = NJ // 8
    Xp = np.zeros((NJ, 142, 142), np.float16)
    Xp[:, 7:135, 7:135] = patches_pairs.astype(np.float16)
    s0, s1, s2 = Xp.strides
    W8 = np.lib.stride_tricks.as_strided(Xp, (NJ, 8, 30, 142), (s0, 16 * s1, s1, s2))
    jobs = W8.reshape(S, 8, 8, 30, 142)  # [s, jj(=4cc+r), blk, i, x]
    xprep = np.zeros((S, 4, 32, 4, 4, 142), np.float16)
    for c in range(4):
        cc, hb = c % 2, c // 2
        xprep[:, :, :30, c, :, :] = jobs[:, 4 * cc:4 * cc + 4, 4 * hb:4 * hb + 4].transpose(0, 1, 3, 2, 4)
    xprep = xprep.reshape(S, 128, 4, 4, 142)

    Wf = kernels_pairs[:, ::-1, ::-1].astype(np.float16)  # [NJ, dy, dx]
    H = np.zeros((NJ, 45, 15), np.float16)
    H[:, 15:30, :] = Wf
    h0, h1, h2 = H.strides
    B = np.lib.stride_tricks.as_strided(H[:, 15:, :], (NJ, 16, 30, 15), (h0, -h1, h1, h2))
    T = np.ascontiguousarray(B.transpose(0, 3, 2, 1))  # [NJ, dx, i, jj] = Wf[i-jj, dx]
    Tj = T.reshape(S, 2, 4, 15, 30, 16)  # [s, cc, r, dx, i, jj]
    tprep = np.zeros((S, 4, 32, 2, 15, 16), np.float16)
    tprep[:, :, :30] = Tj.transpose(0, 2, 4, 1, 3, 5)  # (s, r, i, cc, dx, jj)
    tprep = tprep.reshape(S, 128, 2, 15, 16)
    return xprep, tprep


def _reassemble(res: np.ndarray, NJ: int) -> np.ndarray:
    """res [S, 4c, 16i, 4r, 4b, 128x] fp16 -> [NJ, 128, 128] f32."""
    S = res.shape[0]
    r7 = res.reshape(S, 2, 2, 16, 4, 4, 128)  # (s, hb, cc, i, r, b, x)
    return r7.transpose(0, 2, 4, 1, 5, 3, 6).reshape(NJ, 128, 128).astype(np.float32)


def kernel(patches, kernels, kernel_size, patch_size, fft_size, _collect_results=None):
    """Full inputs in, full output out. Shards BN across 8 cores."""
    from concourse.bass_utils import run_bass_kernel_spmd

    patches = np.asarray(patches)
    kernels = np.asarray(kernels)
    assert patches.shape == (_BN, _C, _P, _P), patches.shape
    assert kernels.shape == (_BN, _C, _K, _K), kernels.shape

    if "nc" not in _nc_cache:
        _nc_cache["nc"] = _build_nc(_SETS_PER_CORE)
    nc = _nc_cache["nc"]

    bn_per_core = _BN // _N_CORES
    in_maps = []
    for core in range(_N_CORES):
        sl = slice(core * bn_per_core, (core + 1) * bn_per_core)
        pp = patches[sl].reshape(-1, _P, _P)
        kp = kernels[sl].reshape(-1, _K, _K)
        xprep, tprep = _host_prep(pp, kp)
        in_maps.append({"xprep": xprep, "tprep": tprep})

    res = run_bass_kernel_spmd(nc, in_maps, core_ids=list(range(_N_CORES)))
    if _collect_results is not None:
        _collect_results.append(res)

    out = np.empty((_BN, _C, _P, _P), dtype=np.float32)
    for core in range(_N_CORES):
        sl = slice(core * bn_per_core, (core + 1) * bn_per_core)
        out[sl] = _reassemble(res.results[core]["out"], _PAIRS_PER_CORE).reshape(
            bn_per_core, _C, _P, _P
        )
    return out
